# revision 1
# baseline (speedup 1.0000x reference)
"""Causal self-attention (B=4, T=2048, C=1024, H=16, D=64, RoPE) on 8 trn2 cores.

Sharding: data-parallel over batch (4) x tensor-parallel over head-halves (2).
core = 2*b + hh handles batch b, heads [hh*8, hh*8+8).

Per-core kernel (all matmuls bf16 with fp32 PSUM accumulation; every matmul
operand at partition base 0 — mixed PE tile positions fault on this setup):
  - QT/KT projection in transposed layout [c_out, t] (lhsT = W column block,
    rhs = x^T), RoPE via PE rotate-permutation matmul + DVE mul/add.
    Tiles hold head pairs: rows 0-63 head 2i, rows 64-127 head 2i+1.
  - V projection in natural layout [t, c_out], stored interleaved with a ones
    column per head (65 cols/head) for free softmax row-sums.
  - scores^T per head pair via ONE K=128 matmul: lhsT = K^T pair chunk
    [128d, 128k], rhs = block-diagonal assembled Q chunk [128, 512]
    (head A rows 0-63 cols 0-255, head B rows 64-127 cols 256-511, zeros
    elsewhere) -> scores^T [128 k, 256 qA | 256 qB].
  - exp on ACT without max subtraction (scores are O(10)); causal masking via
    multiplicative bf16 masks on the two diagonal key chunks.
  - PV: out_h^T accumulated over key chunks with lhsT = V'_h (ones column ->
    row 64 = softmax denominators); per-head psum bank so accumulation
    groups never share a zero region.
  - normalize Y^T by broadcast(1/sums) (PE outer-product), then row-parallel
    output projection -> partial [T, C] fp32 output.
Host sums the two partial outputs of each batch pair.
"""

import os

import numpy as np
import ml_dtypes

import concourse.bass as bass
import concourse.mybir as mybir
import concourse.tile as tile
from concourse.bass_utils import run_bass_kernel_spmd

BF16 = mybir.dt.bfloat16
F32 = mybir.dt.float32
NP_BF16 = ml_dtypes.bfloat16

B, T, C = 4, 2048, 1024
H, D = 16, 64
HPC = 8          # heads per core
CPC = HPC * D    # 512 features per core
N_CORES = 8
QC = 256         # query chunk (scores^T free dim per head)
KC = 128         # key chunk (scores^T partition dim)
NQC = T // QC    # 8 query chunks
ROPE_BASE = 10000.0

LAST_EXEC_NS = None
LAST_RESULTS = None


def _split_sync_waits(nc):
    """This walrus build accepts at most one sync wait per instruction; hoist
    extra waits onto same-engine NOPs inserted immediately before."""
    ctr = 0
    for bb in nc.main_func.blocks:
        insts = bb.instructions
        new = []
        changed = False
        for inst in insts:
            si = inst.sync_info
            waits = list(si.on_wait or []) if si is not None else []
            if len(waits) > 1:
                changed = True
                for w in waits[:-1]:
                    ctr += 1
                    nop = mybir.InstNoOp(
                        name=f"waitsplit_nop_{ctr}", ins=[], outs=[],
                        engine=inst.engine,
                    )
                    nop.sync_info = mybir.SyncInfo(on_wait=[w], on_update=[])
                    new.append(nop)
                inst.sync_info = mybir.SyncInfo(
                    on_wait=[waits[-1]], on_update=list(si.on_update or [])
                )
            new.append(inst)
        if changed:
            insts[:] = new


def _build_nc(split_waits=True, phases=3, attn_g=2, attn_qc=None):
    if attn_qc is None:
        attn_qc = NQC
    nc = bass.Bass()

    xT = nc.dram_tensor("xT", [C, T], BF16, kind="ExternalInput")
    wq = nc.dram_tensor("wq", [C, CPC], BF16, kind="ExternalInput")
    wk = nc.dram_tensor("wk", [C, CPC], BF16, kind="ExternalInput")
    wv = nc.dram_tensor("wv", [C, CPC], BF16, kind="ExternalInput")
    wc = nc.dram_tensor("wc", [CPC, C], BF16, kind="ExternalInput")
    cos2 = nc.dram_tensor("cos2", [128, T], BF16, kind="ExternalInput")
    ssin = nc.dram_tensor("ssin", [128, T], BF16, kind="ExternalInput")
    p128 = nc.dram_tensor("p128", [128, 128], BF16, kind="ExternalInput")
    # multiplicative causal masks for the 4-head-wide probs tile
    mska = nc.dram_tensor("mska", [128, 4 * QC], BF16, kind="ExternalInput")
    mskb = nc.dram_tensor("mskb", [128, 4 * QC], BF16, kind="ExternalInput")
    e2 = nc.dram_tensor("e2", [2, 128], BF16, kind="ExternalInput")
    out = nc.dram_tensor("out", [T, C], F32, kind="ExternalOutput")

    KB = C // 128          # 8 k-blocks over c_in
    NT = CPC // 128        # 4 head-pair tiles for QT/KT/YT
    TT16 = T // 128        # 16 t tiles for V

    with tile.TileContext(nc) as tc:
        with (
            tc.tile_pool(name="singles", bufs=1) as singles,
            tc.tile_pool(name="xw", bufs=1) as xw,
            tc.tile_pool(name="big", bufs=1) as big,
        ):
            # ---- load constants / inputs ----
            cos_sb = singles.tile([128, T], BF16)
            ssin_sb = singles.tile([128, T], BF16)
            p128_sb = singles.tile([128, 128], BF16)
            mska_sb = singles.tile([128, 4 * QC], BF16)
            mskb_sb = singles.tile([128, 4 * QC], BF16)
            e2_sb = singles.tile([2, 128], BF16)
            nc.sync.dma_start(out=cos_sb, in_=cos2[:])
            nc.sync.dma_start(out=ssin_sb, in_=ssin[:])
            nc.sync.dma_start(out=p128_sb, in_=p128[:])
            nc.sync.dma_start(out=mska_sb, in_=mska[:])
            nc.sync.dma_start(out=mskb_sb, in_=mskb[:])
            nc.sync.dma_start(out=e2_sb, in_=e2[:])

            xT_sb = []
            for kb in range(KB):
                t_ = xw.tile([128, T], BF16, name=f"xT{kb}")
                nc.sync.dma_start(out=t_, in_=xT[kb * 128 : (kb + 1) * 128, :])
                xT_sb.append(t_)
            wq_sb, wk_sb, wv_sb = [], [], []
            for nm, dram, lst in (("wq", wq, wq_sb), ("wk", wk, wk_sb), ("wv", wv, wv_sb)):
                for kb in range(KB):
                    t_ = xw.tile([128, CPC], BF16, name=f"{nm}{kb}")
                    nc.sync.dma_start(out=t_, in_=dram[kb * 128 : (kb + 1) * 128, :])
                    lst.append(t_)
            wc_sb = []
            for cb in range(NT):
                t_ = xw.tile([128, C], BF16, name=f"wc{cb}")
                nc.sync.dma_start(out=t_, in_=wc[cb * 128 : (cb + 1) * 128, :])
                wc_sb.append(t_)

            # ---- persistent big tiles ----
            qt_sb = [big.tile([128, T], BF16, name=f"qt{i}") for i in range(NT)]
            kt_sb = [big.tile([128, T], BF16, name=f"kt{i}") for i in range(NT)]
            yt_sb = [big.tile([128, T], BF16, name=f"yt{i}") for i in range(NT)]
            vp_sb = [big.tile([128, HPC * 65], BF16, name=f"vp{tt}") for tt in range(TT16)]
            # sums staging: engine writes land on aligned partitions {0,32,64,96},
            # then small SBUF->SBUF DMAs (no partition alignment rules) regroup.
            stage4 = big.tile([128, 2 * NQC * QC], BF16, name="stage4")
            sums_sb = big.tile([64, QC], BF16, name="sums")
            sinv2_sb = big.tile([2, NT * NQC * QC], BF16, name="sinv2")

            # ---- projections: QT / KT with RoPE ----
            with (
                tc.tile_pool(name="pj_psum", bufs=4, space="PSUM") as pj_psum,
                tc.tile_pool(name="pj_tmp", bufs=4) as pj_tmp,
            ):
                for w_sb, dst in ((wq_sb, qt_sb), (wk_sb, kt_sb)):
                    for i in range(NT):
                        for tc4 in range(T // 512):
                            ts = slice(tc4 * 512, (tc4 + 1) * 512)
                            ps = pj_psum.tile([128, 512], F32, name="pj")
                            for kb in range(KB):
                                nc.tensor.matmul(
                                    ps,
                                    lhsT=w_sb[kb][:, i * 128 : (i + 1) * 128],
                                    rhs=xT_sb[kb][:, ts],
                                    start=(kb == 0),
                                    stop=(kb == KB - 1),
                                )
                            raw = pj_tmp.tile([128, 512], BF16, name="raw")
                            nc.scalar.copy(out=raw, in_=ps)
                            t1 = pj_tmp.tile([128, 512], BF16, name="t1")
                            nc.vector.tensor_mul(t1, raw, cos_sb[:, ts])
                            # rot(q) via partition-shifted single-input ops
                            # (walrus allows shifted bases only for 1-input)
                            rot_sb = pj_tmp.tile([128, 512], BF16, name="rotsb")
                            for rb in (0, 64):
                                nc.vector.tensor_scalar_mul(
                                    rot_sb[rb : rb + 32, :],
                                    raw[rb + 32 : rb + 64, :],
                                    -1.0,
                                )
                                nc.vector.tensor_copy(
                                    out=rot_sb[rb + 32 : rb + 64, :],
                                    in_=raw[rb : rb + 32, :],
                                )
                            t2 = pj_tmp.tile([128, 512], BF16, name="t2")
                            nc.vector.tensor_mul(t2, rot_sb, ssin_sb[:, ts])
                            nc.vector.tensor_add(dst[i][:, ts], t1, t2)

                # ---- V projection into interleaved V' (65 cols/head) ----
                for tt in range(TT16):
                    ps = pj_psum.tile([128, 512], F32, name="pj")
                    for kb in range(KB):
                        nc.tensor.matmul(
                            ps,
                            lhsT=xT_sb[kb][:, tt * 128 : (tt + 1) * 128],
                            rhs=wv_sb[kb][:, :],
                            start=(kb == 0),
                            stop=(kb == KB - 1),
                        )
                    vdst = vp_sb[tt].rearrange("p (h e) -> p h e", e=65)
                    nc.scalar.copy(
                        out=vdst[:, :, 0:64],
                        in_=ps.rearrange("p (h e) -> p h e", e=64),
                    )
                    nc.vector.memset(vdst[:, :, 64:65], 1.0)

            if phases >= 2:
                # ---- attention: scores^T -> exp -> mask -> PV ----
                with (
                    tc.tile_pool(name="st_psum", bufs=2, space="PSUM") as st_psum,
                    tc.tile_pool(name="pv_psum", bufs=1, space="PSUM") as pv_psum,
                    tc.tile_pool(name="probs", bufs=4) as probs_pool,
                    tc.tile_pool(name="qbd", bufs=4) as qbd_pool,
                ):
                    for g in range(attn_g):  # head groups of 4 (pairs 2g, 2g+1)
                        for qc in range(attn_qc):
                            q0 = qc * QC
                            nkc = (qc + 1) * (QC // KC)
                            # block-diagonal Q chunks, one per pair, reused
                            # across all key chunks j
                            qbd = []
                            for pl in range(2):
                                p = 2 * g + pl
                                qb = qbd_pool.tile([128, 2 * QC], BF16, name=f"qbd{pl}")
                                nc.gpsimd.memset(qb[0:64, QC : 2 * QC], 0.0)
                                nc.gpsimd.memset(qb[64:128, 0:QC], 0.0)
                                nc.vector.tensor_copy(
                                    out=qb[0:64, 0:QC], in_=qt_sb[p][0:64, q0 : q0 + QC]
                                )
                                nc.vector.tensor_copy(
                                    out=qb[64:128, QC : 2 * QC],
                                    in_=qt_sb[p][64:128, q0 : q0 + QC],
                                )
                                qbd.append(qb)
                            # one PSUM bank (512 f32) per head so the four
                            # accumulation groups never share a zero region
                            pv = pv_psum.tile([65, 4, 512], F32, name="pv")
                            for j in range(nkc):
                                st = st_psum.tile([128, 4 * QC], F32, name="st")
                                for pl in range(2):
                                    p = 2 * g + pl
                                    nc.tensor.matmul(
                                        st[:, pl * 2 * QC : (pl + 1) * 2 * QC],
                                        lhsT=kt_sb[p][:, j * KC : (j + 1) * KC],
                                        rhs=qbd[pl],
                                        start=True,
                                        stop=True,
                                    )
                                pr = probs_pool.tile([128, 4 * QC], BF16, name="pr")
                                nc.scalar.activation(
                                    out=pr, in_=st,
                                    func=mybir.ActivationFunctionType.Exp, scale=0.125,
                                )
                                if j == nkc - 2:
                                    nc.vector.tensor_mul(pr, pr, mska_sb)
                                elif j == nkc - 1:
                                    nc.vector.tensor_mul(pr, pr, mskb_sb)
                                for hh in range(4):
                                    h = g * 4 + hh
                                    nc.tensor.matmul(
                                        pv[:, hh, 0:QC],
                                        lhsT=vp_sb[j][:, h * 65 : h * 65 + 65],
                                        rhs=pr[:, hh * QC : (hh + 1) * QC],
                                        start=(j == 0),
                                        stop=(j == nkc - 1),
                                    )
                            for hh in range(4):
                                h = g * 4 + hh
                                ro = (h % 2) * 64
                                nc.vector.tensor_copy(
                                    out=yt_sb[h // 2][ro : ro + 64, q0 : q0 + QC],
                                    in_=pv[0:64, hh, 0:QC],
                                )
                                # sums row -> aligned partition 32*(h%4), unique cols
                                sp = 32 * (h % 4)
                                sc = ((h // 4) * NQC + qc) * QC
                                nc.vector.tensor_copy(
                                    out=stage4[sp : sp + 1, sc : sc + QC],
                                    in_=pv[64:65, hh, 0:QC],
                                )
                                r = qc * 8 + h
                                nc.sync.dma_start(
                                    out=sums_sb[r : r + 1, :],
                                    in_=stage4[sp : sp + 1, sc : sc + QC],
                                )

            if phases >= 3:
                # ---- normalize Y^T and output projection ----
                with (
                    tc.tile_pool(name="bc_psum", bufs=2, space="PSUM") as bc_psum,
                    tc.tile_pool(name="o_psum", bufs=2, space="PSUM") as o_psum,
                    tc.tile_pool(name="o_tmp", bufs=4) as o_tmp,
                    tc.tile_pool(name="sinvp", bufs=1) as sinvp,
                ):
                    sinv_sb = sinvp.tile([64, QC], BF16)
                    with nc.allow_low_precision(reason="softmax denominators tolerate bf16"):
                        nc.vector.reciprocal(out=sinv_sb, in_=sums_sb)
                    # scatter [64, QC] rows (qc*8 + 2i + p) -> [2, (i*NQC+qc)*QC + c]
                    for i in range(NT):
                        for qc in range(NQC):
                            r = qc * 8 + 2 * i
                            s0 = (i * NQC + qc) * QC
                            nc.sync.dma_start(
                                out=sinv2_sb[0:2, s0 : s0 + QC],
                                in_=sinv_sb[r : r + 2, :],
                            )
                    for i in range(NT):
                        for qc in range(NQC):
                            bc = bc_psum.tile([128, QC], F32, name="bc")
                            s0 = (i * NQC + qc) * QC
                            nc.tensor.matmul(
                                bc, lhsT=e2_sb, rhs=sinv2_sb[0:2, s0 : s0 + QC],
                                start=True, stop=True,
                            )
                            bcs = o_tmp.tile([128, QC], BF16, name="bcs")
                            nc.vector.tensor_copy(out=bcs, in_=bc)
                            ts = slice(qc * QC, (qc + 1) * QC)
                            nc.vector.tensor_mul(yt_sb[i][:, ts], yt_sb[i][:, ts], bcs)

                    for qt in range(TT16):
                        for co in range(2):
                            ps = o_psum.tile([128, 512], F32, name="op")
                            for cb in range(NT):
                                nc.tensor.matmul(
                                    ps,
                                    lhsT=yt_sb[cb][:, qt * 128 : (qt + 1) * 128],
                                    rhs=wc_sb[cb][:, co * 512 : (co + 1) * 512],
                                    start=(cb == 0),
                                    stop=(cb == NT - 1),
                                )
                            st_ = o_tmp.tile([128, 512], F32, name="ost")
                            nc.scalar.copy(out=st_, in_=ps)
                            nc.sync.dma_start(
                                out=out[qt * 128 : (qt + 1) * 128, co * 512 : (co + 1) * 512],
                                in_=st_,
                            )
    if split_waits:
        _split_sync_waits(nc)
    return nc


_NC = None


def _host_tables():
    inv_freq = 1.0 / (ROPE_BASE ** (np.arange(0, D, 2, dtype=np.float32) / D))
    t = np.arange(T, dtype=np.float32)
    freqs = np.einsum("i,j->ij", t, inv_freq)          # [T, 32]
    emb = np.concatenate([freqs, freqs], axis=-1)      # [T, 64]
    cosT = np.cos(emb).T.astype(np.float32)            # [64, T]
    sinT = np.sin(emb).T.astype(np.float32)
    cos2 = np.concatenate([cosT, cosT], axis=0)        # [128, T]
    ssin = np.concatenate([sinT, sinT], axis=0)        # [128, T]

    # rotate-half permutation as matmul lhsT: out[m] = sum_k P[k, m] * in[k]
    p128 = np.zeros((128, 128), dtype=np.float32)
    for blk in (0, 64):
        for m in range(32):
            p128[blk + m + 32, blk + m] = -1.0      # out[m] = -in[m+32]
            p128[blk + m, blk + m + 32] = 1.0       # out[m+32] = in[m]

    # causal masks on probs^T [128 keys, QC queries], replicated for 4 heads
    i_ = np.arange(KC)[:, None]
    c_ = np.arange(QC)[None, :]
    mska1 = (c_ >= i_).astype(np.float32)           # key chunk aligned at q0
    mskb1 = (c_ >= i_ + 128).astype(np.float32)     # key chunk at q0+128
    mska = np.tile(mska1, (1, 4))
    mskb = np.tile(mskb1, (1, 4))

    e2 = np.zeros((2, 128), dtype=np.float32)
    e2[0, 0:64] = 1.0
    e2[1, 64:128] = 1.0
    return cos2, ssin, p128, mska, mskb, e2


def kernel(x, Wq, Wkv, Wc):
    global _NC, LAST_EXEC_NS, LAST_RESULTS
    x = np.asarray(x, dtype=np.float32)
    Wq = np.asarray(Wq, dtype=np.float32)
    Wkv = np.asarray(Wkv, dtype=np.float32)
    Wc = np.asarray(Wc, dtype=np.float32)

    if _NC is None:
        _NC = _build_nc()

    cos2, ssin, p128, mska, mskb, e2 = _host_tables()
    bf = lambda a: np.ascontiguousarray(a).astype(NP_BF16)

    in_maps = []
    for core in range(N_CORES):
        b, hh = core // 2, core % 2
        h0 = hh * HPC
        cols = slice(h0 * D, h0 * D + CPC)
        vcols = slice(C + h0 * D, C + h0 * D + CPC)
        in_maps.append(
            {
                "xT": bf(x[b].T),
                "wq": bf(Wq[:, cols]),
                "wk": bf(Wkv[:, cols]),
                "wv": bf(Wkv[:, vcols]),
                "wc": bf(Wc[cols.start : cols.stop, :]),
                "cos2": bf(cos2),
                "ssin": bf(ssin),
                "p128": bf(p128),
                "mska": bf(mska),
                "mskb": bf(mskb),
                "e2": bf(e2),
            }
        )

    trace = os.environ.get("BASS_PROF", "0") == "1"
    res = run_bass_kernel_spmd(_NC, in_maps, list(range(N_CORES)), trace=trace)
    LAST_EXEC_NS = res.exec_time_ns
    LAST_RESULTS = res
    y = np.empty((B, T, C), dtype=np.float32)
    for b in range(B):
        y[b] = res.results[2 * b]["out"] + res.results[2 * b + 1]["out"]
    return y



# revision 6
# speedup vs baseline: 1.4460x; 1.4460x over previous
"""Causal self-attention (B=4, T=2048, C=1024, H=16, D=64, RoPE) on 8 trn2 cores.

Sharding: data-parallel over batch (4) x tensor-parallel over head-halves (2).
core = 2*b + hh handles batch b, heads [hh*8, hh*8+8).

Per-core kernel (all matmuls bf16 with fp32 PSUM accumulation; every matmul
operand at partition base 0 — mixed PE tile positions fault on this setup):

  - QT/KT projection in transposed layout [c_out, t] (lhsT = W column block,
    rhs = x^T), RoPE via partition-shifted 1-input DVE ops + mul/add.
    Tiles hold head pairs: rows 0-63 head 2p, rows 64-127 head 2p+1.
    RoPE'd Q is stored straight into a block-diagonal layout qbd[p]
    [128, 2, T]: block 0 rows 0-63 = head 2p (rows 64-127 zero), block 1
    rows 64-127 = head 2p+1 — scores rhs slices come from here for free.
  - V projection in natural layout [t, c_out], stored interleaved with a ones
    column per head (65 cols/head) for free softmax row-sums.
  - scores^T per (head-group g of 4, q-chunk qq of 128): batches of 2 key
    chunks land in one PSUM tile [128k, 2, 2x2x128q]; one K=128 matmul per
    (key chunk, pair) with the block-diag q slice as rhs. exp on ACT over the
    whole [128, 1024] batch (scale=0.125, no max subtraction; scores are
    O(10)); causal masking via one multiplicative bf16 mask on the diagonal
    key chunk only.
  - PV TRANSPOSED: lhsT = probs chunk [128k, 128q] (stationary), rhs = V'_h
    [128k, 65] (streamed, ones col -> col 64 = softmax denominators), out
    accumulates [128q, 65] per head over key chunks. All 4 heads of a group
    pack into ONE psum bank (single start=True on the first matmul; each
    span is first-touched exactly once while pending-zero).
  - normalize while copying out of PSUM: per-partition reciprocal of the
    denominator column, then tensor_scalar_mul psum->sbuf into y natural
    layout [t, 512].
  - y^T via PE transposes (is_transpose matmuls vs an identity, bf16 psum
    out), then row-parallel output projection -> partial [T, C] fp32 output.
Host sums the two partial outputs of each batch pair.

Phase overlap: QK proj g0 -> attention g0 (ACT exp-bound) overlapped with
QK proj g1 + V proj on PE -> attention g1 overlapped with transposes +
output projection of the previous q-chunk.
"""

import os

import numpy as np
import ml_dtypes

import concourse.bass as bass
import concourse.mybir as mybir
import concourse.tile as tile
from concourse.bass_utils import run_bass_kernel_spmd

BF16 = mybir.dt.bfloat16
F32 = mybir.dt.float32
NP_BF16 = ml_dtypes.bfloat16

B, T, C = 4, 2048, 1024
H, D = 16, 64
HPC = 8          # heads per core
CPC = HPC * D    # 512 features per core
N_CORES = 8
KC = 128         # key chunk
NQ = T // KC     # 16 q-chunks of 128
ROPE_BASE = 10000.0

LAST_EXEC_NS = None
LAST_RESULTS = None


def _split_sync_waits(nc):
    """This walrus build accepts at most one sync wait per instruction; hoist
    extra waits onto same-engine NOPs inserted immediately before."""
    ctr = 0
    for bb in nc.main_func.blocks:
        insts = bb.instructions
        new = []
        changed = False
        for inst in insts:
            si = inst.sync_info
            waits = list(si.on_wait or []) if si is not None else []
            if len(waits) > 1:
                changed = True
                for w in waits[:-1]:
                    ctr += 1
                    nop = mybir.InstNoOp(
                        name=f"waitsplit_nop_{ctr}", ins=[], outs=[],
                        engine=inst.engine,
                    )
                    nop.sync_info = mybir.SyncInfo(on_wait=[w], on_update=[])
                    new.append(nop)
                inst.sync_info = mybir.SyncInfo(
                    on_wait=[waits[-1]], on_update=list(si.on_update or [])
                )
            new.append(inst)
        if changed:
            insts[:] = new


def _build_nc(split_waits=True):
    nc = bass.Bass()

    xT = nc.dram_tensor("xT", [C, T], BF16, kind="ExternalInput")
    wq = nc.dram_tensor("wq", [C, CPC], BF16, kind="ExternalInput")
    wk = nc.dram_tensor("wk", [C, CPC], BF16, kind="ExternalInput")
    wv = nc.dram_tensor("wv", [C, CPC], BF16, kind="ExternalInput")
    wc = nc.dram_tensor("wc", [CPC, C], BF16, kind="ExternalInput")
    cos2 = nc.dram_tensor("cos2", [128, T], BF16, kind="ExternalInput")
    ssin = nc.dram_tensor("ssin", [128, T], BF16, kind="ExternalInput")
    # triangular causal mask for the diagonal key chunk, replicated 4 heads
    maskd = nc.dram_tensor("maskd", [128, 4 * KC], BF16, kind="ExternalInput")
    ident = nc.dram_tensor("ident", [128, 128], BF16, kind="ExternalInput")
    out = nc.dram_tensor("out", [T, C], F32, kind="ExternalOutput")

    KB = C // 128          # 8 k-blocks over c_in
    NT = CPC // 128        # 4 head-pair tiles
    TT16 = T // 128        # 16 t tiles

    with tile.TileContext(nc) as tc:
        with (
            tc.tile_pool(name="singles", bufs=1) as singles,
            tc.tile_pool(name="xw", bufs=1) as xw,
            tc.tile_pool(name="big", bufs=1) as big,
            tc.tile_pool(name="rope", bufs=2) as rope_pool,
            tc.tile_pool(name="probs", bufs=4) as probs_pool,
            tc.tile_pool(name="normp", bufs=4) as norm_pool,
            tc.tile_pool(name="ytcp", bufs=2) as ytc_pool,
            tc.tile_pool(name="ostp", bufs=3) as ost_pool,
            tc.tile_pool(name="scratch", bufs=3, space="PSUM") as scratch,
            tc.tile_pool(name="stp", bufs=2, space="PSUM") as stp,
            tc.tile_pool(name="pvp", bufs=1, space="PSUM") as pvp,
        ):
            # ---- input DMAs (interleaved so phase A can start early) ----
            xT_sb, wq_sb, wk_sb, wv_sb = [], [], [], []
            for kb in range(KB):
                t_ = xw.tile([128, CPC], BF16, name=f"wq{kb}")
                nc.sync.dma_start(out=t_, in_=wq[kb * 128 : (kb + 1) * 128, :])
                wq_sb.append(t_)
                t_ = xw.tile([128, T], BF16, name=f"xT{kb}")
                nc.sync.dma_start(out=t_, in_=xT[kb * 128 : (kb + 1) * 128, :])
                xT_sb.append(t_)
                t_ = xw.tile([128, CPC], BF16, name=f"wk{kb}")
                nc.sync.dma_start(out=t_, in_=wk[kb * 128 : (kb + 1) * 128, :])
                wk_sb.append(t_)
            cos_sb = singles.tile([128, T], BF16)
            ssin_sb = singles.tile([128, T], BF16)
            ident_sb = singles.tile([128, 128], BF16)
            maskd_sb = singles.tile([128, 4 * KC], BF16)
            nc.sync.dma_start(out=cos_sb, in_=cos2[:])
            nc.sync.dma_start(out=ssin_sb, in_=ssin[:])
            nc.sync.dma_start(out=ident_sb, in_=ident[:])
            for kb in range(KB):
                t_ = xw.tile([128, CPC], BF16, name=f"wv{kb}")
                nc.sync.dma_start(out=t_, in_=wv[kb * 128 : (kb + 1) * 128, :])
                wv_sb.append(t_)
            wc_sb = []
            for cb in range(NT):
                t_ = xw.tile([128, C], BF16, name=f"wc{cb}")
                nc.sync.dma_start(out=t_, in_=wc[cb * 128 : (cb + 1) * 128, :])
                wc_sb.append(t_)
            nc.sync.dma_start(out=maskd_sb, in_=maskd[:])

            # ---- persistent tiles ----
            qbd = [big.tile([128, 2, T], BF16, name=f"qbd{p}") for p in range(NT)]
            kt_sb = [big.tile([128, T], BF16, name=f"kt{p}") for p in range(NT)]
            vp_sb = [big.tile([128, HPC, 65], BF16, name=f"vp{tt}") for tt in range(TT16)]
            ynat = [big.tile([128, CPC], BF16, name=f"yn{tt}") for tt in range(TT16)]

            # zero the off-diagonal halves of the block-diag q tiles
            for p in range(NT):
                nc.gpsimd.memset(qbd[p][64:128, 0, :], 0.0)
                nc.gpsimd.memset(qbd[p][0:64, 1, :], 0.0)

            w_map = {"q": wq_sb, "k": wk_sb}

            def qk_unit(p, kind, tc4):
                """Project one 512-col chunk of Q^T or K^T for pair p, RoPE it."""
                ts = slice(tc4 * 512, (tc4 + 1) * 512)
                ps = scratch.tile([128, 512], F32, name="pj", tag="s")
                w_sb = w_map[kind]
                for kb in range(KB):
                    nc.tensor.matmul(
                        ps,
                        lhsT=w_sb[kb][:, p * 128 : (p + 1) * 128],
                        rhs=xT_sb[kb][:, ts],
                        start=(kb == 0),
                        stop=(kb == KB - 1),
                    )
                raw = rope_pool.tile([128, 512], BF16, name="raw")
                nc.scalar.copy(out=raw, in_=ps)
                t1 = rope_pool.tile([128, 512], BF16, name="t1")
                nc.vector.tensor_mul(t1, raw, cos_sb[:, ts])
                # rotate-half via partition-shifted single-input ops
                # (walrus allows shifted bases only for 1-input)
                rot = rope_pool.tile([128, 512], BF16, name="rot")
                for rb in (0, 64):
                    nc.vector.tensor_scalar_mul(
                        rot[rb : rb + 32, :], raw[rb + 32 : rb + 64, :], -1.0
                    )
                    nc.vector.tensor_copy(
                        out=rot[rb + 32 : rb + 64, :], in_=raw[rb : rb + 32, :]
                    )
                t2 = rope_pool.tile([128, 512], BF16, name="t2")
                nc.vector.tensor_mul(t2, rot, ssin_sb[:, ts])
                if kind == "k":
                    nc.vector.tensor_add(kt_sb[p][:, ts], t1, t2)
                else:
                    qt = rope_pool.tile([128, 512], BF16, name="qt")
                    nc.vector.tensor_add(qt, t1, t2)
                    nc.vector.tensor_copy(out=qbd[p][0:64, 0, ts], in_=qt[0:64, :])
                    nc.vector.tensor_copy(out=qbd[p][64:128, 1, ts], in_=qt[64:128, :])

            def v_unit(tt):
                """Project V for t-chunk tt into interleaved V' (65 cols/head)."""
                ps = scratch.tile([128, 512], F32, name="pj", tag="s")
                for kb in range(KB):
                    nc.tensor.matmul(
                        ps,
                        lhsT=xT_sb[kb][:, tt * 128 : (tt + 1) * 128],
                        rhs=wv_sb[kb][:, :],
                        start=(kb == 0),
                        stop=(kb == KB - 1),
                    )
                nc.vector.tensor_copy(
                    out=vp_sb[tt][:, :, 0:64],
                    in_=ps.rearrange("p (h e) -> p h e", e=64),
                )
                nc.vector.memset(vp_sb[tt][:, :, 64:65], 1.0)

            def attention(g, qq):
                """Scores^T -> exp -> mask -> transposed PV -> normalized y."""
                nb = (qq + 2) // 2
                qsl = slice(qq * 128, (qq + 1) * 128)
                pv = pvp.tile([128, 4, 128], F32, name="pv")
                n_pv = 0
                tot_pv = (qq + 1) * 4
                for b in range(nb):
                    full = (2 * b + 1) <= qq
                    jsubs = (0, 1) if full else (0,)
                    st = stp.tile([128, 2, 512], F32, name="st")
                    for jsub in jsubs:
                        j = 2 * b + jsub
                        for pl in range(2):
                            p = 2 * g + pl
                            nc.tensor.matmul(
                                st[:, jsub, pl * 256 : (pl + 1) * 256],
                                lhsT=kt_sb[p][:, j * KC : (j + 1) * KC],
                                rhs=qbd[p][:, :, qsl],
                                start=(pl == 0),
                                stop=(pl == 1),
                            )
                    pr = probs_pool.tile([128, 2, 512], BF16, name="pr")
                    if full:
                        nc.scalar.activation(
                            out=pr, in_=st,
                            func=mybir.ActivationFunctionType.Exp, scale=0.125,
                        )
                    else:
                        nc.scalar.activation(
                            out=pr[:, 0, :], in_=st[:, 0, :],
                            func=mybir.ActivationFunctionType.Exp, scale=0.125,
                        )
                    if 2 * b <= qq <= 2 * b + 1:
                        jd = qq - 2 * b
                        nc.vector.tensor_mul(pr[:, jd, :], pr[:, jd, :], maskd_sb)
                    for jsub in jsubs:
                        j = 2 * b + jsub
                        for hh in range(4):
                            h = 4 * g + hh
                            n_pv += 1
                            nc.tensor.matmul(
                                pv[:, hh, 0:65],
                                lhsT=pr[:, jsub, hh * 128 : (hh + 1) * 128],
                                rhs=vp_sb[j][:, h, :],
                                start=(n_pv == 1),
                                stop=(n_pv == tot_pv),
                            )
                rcp = norm_pool.tile([128, 4, 1], F32, name="rcp")
                nc.vector.reciprocal(out=rcp, in_=pv[:, :, 64:65])
                for hh in range(4):
                    h = 4 * g + hh
                    nc.vector.tensor_scalar_mul(
                        ynat[qq][:, h * 64 : (h + 1) * 64],
                        pv[:, hh, 0:64],
                        rcp[:, hh, :],
                    )

            def tails(tt):
                """Transpose y chunk tt, project through Wc, DMA out."""
                tp = scratch.tile([128, 4, 128], BF16, name="tp", tag="s")
                for c in range(NT):
                    nc.tensor.matmul(
                        tp[:, c, :],
                        lhsT=ynat[tt][:, c * 128 : (c + 1) * 128],
                        rhs=ident_sb,
                        is_transpose=True,
                        start=(c == 0),
                        stop=(c == NT - 1),
                    )
                ytc = ytc_pool.tile([128, 4, 128], BF16, name="ytc")
                nc.vector.tensor_copy(out=ytc, in_=tp)
                for co in range(2):
                    op = scratch.tile([128, 512], F32, name="op", tag="s")
                    for cb in range(NT):
                        nc.tensor.matmul(
                            op,
                            lhsT=ytc[:, cb, :],
                            rhs=wc_sb[cb][:, co * 512 : (co + 1) * 512],
                            start=(cb == 0),
                            stop=(cb == NT - 1),
                        )
                    ost = ost_pool.tile([128, 512], F32, name="ost")
                    nc.vector.tensor_copy(out=ost, in_=op)
                    nc.sync.dma_start(
                        out=out[tt * 128 : (tt + 1) * 128, co * 512 : (co + 1) * 512],
                        in_=ost,
                    )

            # ---- phase A: QK proj for head group 0 + first V chunk ----
            for p in (0, 1):
                for kind in ("q", "k"):
                    for tc4 in range(4):
                        qk_unit(p, kind, tc4)
            v_unit(0)

            # ---- phase B: attention g0, with QK g1 + V proj interleaved ----
            g1_units = [
                (p, kind, tc4)
                for p in (2, 3)
                for kind in ("q", "k")
                for tc4 in range(4)
            ]
            for qq in range(NQ):
                if qq + 1 < TT16:
                    v_unit(qq + 1)
                qk_unit(*g1_units[qq])
                attention(0, qq)

            # ---- phase C: attention g1 + transpose/output-proj (lag 1) ----
            for qq in range(NQ):
                attention(1, qq)
                if qq >= 1:
                    tails(qq - 1)
            tails(NQ - 1)

    if split_waits:
        _split_sync_waits(nc)
    return nc


_NC = None


def _host_tables():
    inv_freq = 1.0 / (ROPE_BASE ** (np.arange(0, D, 2, dtype=np.float32) / D))
    t = np.arange(T, dtype=np.float32)
    freqs = np.einsum("i,j->ij", t, inv_freq)          # [T, 32]
    emb = np.concatenate([freqs, freqs], axis=-1)      # [T, 64]
    cosT = np.cos(emb).T.astype(np.float32)            # [64, T]
    sinT = np.sin(emb).T.astype(np.float32)
    cos2 = np.concatenate([cosT, cosT], axis=0)        # [128, T]
    ssin = np.concatenate([sinT, sinT], axis=0)        # [128, T]

    # causal mask on the diagonal key chunk, probs^T layout [128 keys, 128 q],
    # replicated for 4 heads
    i_ = np.arange(KC)[:, None]
    c_ = np.arange(KC)[None, :]
    m1 = (c_ >= i_).astype(np.float32)
    maskd = np.tile(m1, (1, 4))

    ident = np.eye(128, dtype=np.float32)
    return cos2, ssin, maskd, ident


def kernel(x, Wq, Wkv, Wc):
    global _NC, LAST_EXEC_NS, LAST_RESULTS
    x = np.asarray(x, dtype=np.float32)
    Wq = np.asarray(Wq, dtype=np.float32)
    Wkv = np.asarray(Wkv, dtype=np.float32)
    Wc = np.asarray(Wc, dtype=np.float32)

    if _NC is None:
        _NC = _build_nc()

    cos2, ssin, maskd, ident = _host_tables()
    bf = lambda a: np.ascontiguousarray(a).astype(NP_BF16)

    in_maps = []
    for core in range(N_CORES):
        b, hh = core // 2, core % 2
        h0 = hh * HPC
        cols = slice(h0 * D, h0 * D + CPC)
        vcols = slice(C + h0 * D, C + h0 * D + CPC)
        in_maps.append(
            {
                "xT": bf(x[b].T),
                "wq": bf(Wq[:, cols]),
                "wk": bf(Wkv[:, cols]),
                "wv": bf(Wkv[:, vcols]),
                "wc": bf(Wc[cols.start : cols.stop, :]),
                "cos2": bf(cos2),
                "ssin": bf(ssin),
                "maskd": bf(maskd),
                "ident": bf(ident),
            }
        )

    trace = os.environ.get("BASS_PROF", "0") == "1"
    res = run_bass_kernel_spmd(_NC, in_maps, list(range(N_CORES)), trace=trace)
    LAST_EXEC_NS = res.exec_time_ns
    LAST_RESULTS = res
    y = np.empty((B, T, C), dtype=np.float32)
    for b in range(B):
        y[b] = res.results[2 * b]["out"] + res.results[2 * b + 1]["out"]
    return y


# revision 16
# speedup vs baseline: 1.4597x; 1.0095x over previous
"""Causal self-attention (B=4, T=2048, C=1024, H=16, D=64, RoPE) on 8 trn2 cores.

Sharding: data-parallel over batch (4) x tensor-parallel over head-halves (2).
core = 2*b + hh handles batch b, heads [hh*8, hh*8+8).

Per-core kernel (all matmuls bf16 with fp32 PSUM accumulation; every matmul
operand at partition base 0 — mixed PE tile positions fault on this setup):

  - QT/KT projection in transposed layout [c_out, t] (lhsT = W column block,
    rhs = x^T), RoPE via partition-shifted 1-input DVE ops + mul/add.
    Tiles hold head pairs: rows 0-63 head 2p, rows 64-127 head 2p+1.
    RoPE'd Q is stored straight into a block-diagonal layout qbd[p]
    [128, 2, T]: block 0 rows 0-63 = head 2p (rows 64-127 zero), block 1
    rows 64-127 = head 2p+1 — scores rhs slices come from here for free.
  - V projection in natural layout [t, c_out], stored interleaved with a ones
    column per head (65 cols/head) for free softmax row-sums.
  - scores^T per (head-group g of 4, q-chunk qq of 128): batches of 2 key
    chunks land in one PSUM tile [128k, 2, 2x2x128q]; one K=128 matmul per
    (key chunk, pair) with the block-diag q slice as rhs. exp on ACT over the
    whole [128, 1024] batch (scale=0.125, no max subtraction; scores are
    O(10)); causal masking via one multiplicative bf16 mask on the diagonal
    key chunk only.
  - PV TRANSPOSED: lhsT = probs chunk [128k, 128q] (stationary), rhs = V'_h
    [128k, 65] (streamed, ones col -> col 64 = softmax denominators), out
    accumulates [128q, 65] per head over key chunks. All 4 heads of a group
    pack into ONE psum bank (single start=True on the first matmul; each
    span is first-touched exactly once while pending-zero).
  - normalize while copying out of PSUM: per-partition reciprocal of the
    denominator column, then tensor_scalar_mul psum->sbuf into y natural
    layout [t, 512].
  - y^T via PE transposes (is_transpose matmuls vs an identity, bf16 psum
    out), then row-parallel output projection -> partial [T, C] fp32 output.
Host sums the two partial outputs of each batch pair.

Phase overlap: QK proj g0 -> attention g0 (ACT exp-bound) overlapped with
QK proj g1 + V proj on PE -> attention g1 overlapped with transposes +
output projection of the previous q-chunk.
"""

import os

import numpy as np
import ml_dtypes

import concourse.bass as bass
import concourse.mybir as mybir
import concourse.tile as tile
from concourse.bass_utils import run_bass_kernel_spmd

BF16 = mybir.dt.bfloat16
F32 = mybir.dt.float32
NP_BF16 = ml_dtypes.bfloat16

B, T, C = 4, 2048, 1024
H, D = 16, 64
HPC = 8          # heads per core
CPC = HPC * D    # 512 features per core
N_CORES = 8
KC = 128         # key chunk
NQ = T // KC     # 16 q-chunks of 128
ROPE_BASE = 10000.0

LAST_EXEC_NS = None
LAST_RESULTS = None


def _split_sync_waits(nc):
    """This walrus build accepts at most one sync wait per instruction; hoist
    extra waits onto same-engine NOPs inserted immediately before."""
    ctr = 0
    for bb in nc.main_func.blocks:
        insts = bb.instructions
        new = []
        changed = False
        for inst in insts:
            si = inst.sync_info
            waits = list(si.on_wait or []) if si is not None else []
            if len(waits) > 1:
                changed = True
                for w in waits[:-1]:
                    ctr += 1
                    nop = mybir.InstNoOp(
                        name=f"waitsplit_nop_{ctr}", ins=[], outs=[],
                        engine=inst.engine,
                    )
                    nop.sync_info = mybir.SyncInfo(on_wait=[w], on_update=[])
                    new.append(nop)
                inst.sync_info = mybir.SyncInfo(
                    on_wait=[waits[-1]], on_update=list(si.on_update or [])
                )
            new.append(inst)
        if changed:
            insts[:] = new


def _build_nc(split_waits=True):
    nc = bass.Bass()

    xT = nc.dram_tensor("xT", [C, T], BF16, kind="ExternalInput")
    wqkv = nc.dram_tensor("wqkv", [C, 3 * CPC], BF16, kind="ExternalInput")
    wc = nc.dram_tensor("wc", [CPC, C], BF16, kind="ExternalInput")
    # tables = [cos2 | ssin | maskd | ident], all [128, .]
    tables = nc.dram_tensor("tables", [128, 2 * T + 4 * KC + 128], BF16,
                            kind="ExternalInput")
    out = nc.dram_tensor("out", [T, C], F32, kind="ExternalOutput")

    KB = C // 128          # 8 k-blocks over c_in
    NT = CPC // 128        # 4 head-pair tiles
    TT16 = T // 128        # 16 t tiles

    with tile.TileContext(nc) as tc:
        with (
            tc.tile_pool(name="singles", bufs=1) as singles,
            tc.tile_pool(name="xw", bufs=1) as xw,
            tc.tile_pool(name="big", bufs=1) as big,
            tc.tile_pool(name="rope", bufs=2) as rope_pool,
            tc.tile_pool(name="probs", bufs=4) as probs_pool,
            tc.tile_pool(name="normp", bufs=4) as norm_pool,
            tc.tile_pool(name="ytcp", bufs=2) as ytc_pool,
            tc.tile_pool(name="ostp", bufs=3) as ost_pool,
            tc.tile_pool(name="scratch", bufs=3, space="PSUM") as scratch,
            tc.tile_pool(name="stp", bufs=2, space="PSUM") as stp,
            tc.tile_pool(name="pvp", bufs=1, space="PSUM") as pvp,
        ):
            # ---- input DMAs: what phase A needs first (wq/xT/wk), then
            # tables, then wv/wc which are consumed later ----
            xT_sb, wq_sb, wk_sb, wv_sb = [], [], [], []
            for kb in range(KB):
                rows = slice(kb * 128, (kb + 1) * 128)
                t_ = xw.tile([128, 3 * CPC], BF16, name=f"wqkv{kb}")
                nc.sync.dma_start(out=t_[:, 0:CPC], in_=wqkv[rows, 0:CPC])
                wq_sb.append(t_[:, 0:CPC])
                wk_sb.append(t_[:, CPC : 2 * CPC])
                wv_sb.append(t_[:, 2 * CPC : 3 * CPC])
                x_ = xw.tile([128, T], BF16, name=f"xT{kb}")
                nc.sync.dma_start(out=x_, in_=xT[rows, :])
                xT_sb.append(x_)
                nc.sync.dma_start(
                    out=wk_sb[kb], in_=wqkv[rows, CPC : 2 * CPC]
                )
            tbl = singles.tile([128, 2 * T + 4 * KC + 128], BF16, name="tbl")
            nc.sync.dma_start(out=tbl, in_=tables[:])
            for kb in range(KB):
                rows = slice(kb * 128, (kb + 1) * 128)
                nc.sync.dma_start(
                    out=wv_sb[kb], in_=wqkv[rows, 2 * CPC : 3 * CPC]
                )
            cos_sb = tbl[:, 0:T]
            ssin_sb = tbl[:, T : 2 * T]
            maskd_sb = tbl[:, 2 * T : 2 * T + 4 * KC]
            ident_sb = tbl[:, 2 * T + 4 * KC : 2 * T + 4 * KC + 128]
            wcbig = xw.tile([128, NT, C], BF16, name="wcbig")
            nc.sync.dma_start(
                out=wcbig,
                in_=wc.rearrange("(cb p) c -> p cb c", p=128),
            )
            wc_sb = [wcbig[:, cb, :] for cb in range(NT)]

            # ---- persistent tiles ----
            qbd = [big.tile([128, 2, T], BF16, name=f"qbd{p}") for p in range(NT)]
            kt_sb = [big.tile([128, T], BF16, name=f"kt{p}") for p in range(NT)]
            vp_sb = [big.tile([128, HPC, 65], BF16, name=f"vp{tt}") for tt in range(TT16)]
            ynat = [big.tile([128, CPC], BF16, name=f"yn{tt}") for tt in range(TT16)]

            # zero the off-diagonal halves of the block-diag q tiles
            for p in range(NT):
                nc.gpsimd.memset(qbd[p][64:128, 0, :], 0.0)
                nc.gpsimd.memset(qbd[p][0:64, 1, :], 0.0)

            w_map = {"q": wq_sb, "k": wk_sb}

            def qk_unit(p, kind, tc4, rot_on_pool=False):
                """Project one 512-col chunk of Q^T or K^T for pair p, RoPE it."""
                ts = slice(tc4 * 512, (tc4 + 1) * 512)
                ps = scratch.tile([128, 512], F32, name="pj", tag="s")
                w_sb = w_map[kind]
                for kb in range(KB):
                    nc.tensor.matmul(
                        ps,
                        lhsT=w_sb[kb][:, p * 128 : (p + 1) * 128],
                        rhs=xT_sb[kb][:, ts],
                        start=(kb == 0),
                        stop=(kb == KB - 1),
                    )
                raw = rope_pool.tile([128, 512], BF16, name="raw")
                nc.scalar.copy(out=raw, in_=ps)
                t1 = rope_pool.tile([128, 512], BF16, name="t1")
                nc.vector.tensor_mul(t1, raw, cos_sb[:, ts])
                # rotate-half via partition-shifted single-input ops
                # (walrus allows shifted bases only for 1-input); the units
                # that overlap attention run these on the idle Pool engine
                rot_eng = nc.gpsimd if rot_on_pool else nc.vector
                rot = rope_pool.tile([128, 512], BF16, name="rot")
                for rb in (0, 64):
                    rot_eng.tensor_scalar_mul(
                        rot[rb : rb + 32, :], raw[rb + 32 : rb + 64, :], -1.0
                    )
                    rot_eng.tensor_copy(
                        out=rot[rb + 32 : rb + 64, :], in_=raw[rb : rb + 32, :]
                    )
                t2 = rope_pool.tile([128, 512], BF16, name="t2")
                nc.vector.tensor_mul(t2, rot, ssin_sb[:, ts])
                if kind == "k":
                    nc.vector.tensor_add(kt_sb[p][:, ts], t1, t2)
                else:
                    qt = rope_pool.tile([128, 512], BF16, name="qt")
                    nc.vector.tensor_add(qt, t1, t2)
                    nc.vector.tensor_copy(out=qbd[p][0:64, 0, ts], in_=qt[0:64, :])
                    nc.vector.tensor_copy(out=qbd[p][64:128, 1, ts], in_=qt[64:128, :])

            def v_unit(tt):
                """Project V for t-chunk tt into interleaved V' (65 cols/head)."""
                ps = scratch.tile([128, 512], F32, name="pj", tag="s")
                for kb in range(KB):
                    nc.tensor.matmul(
                        ps,
                        lhsT=xT_sb[kb][:, tt * 128 : (tt + 1) * 128],
                        rhs=wv_sb[kb][:, :],
                        start=(kb == 0),
                        stop=(kb == KB - 1),
                    )
                nc.vector.tensor_copy(
                    out=vp_sb[tt][:, :, 0:64],
                    in_=ps.rearrange("p (h e) -> p h e", e=64),
                )
                nc.vector.memset(vp_sb[tt][:, :, 64:65], 1.0)

            def attention(g, qq):
                """Scores^T -> exp -> mask -> transposed PV -> normalized y."""
                nb = (qq + 2) // 2
                qsl = slice(qq * 128, (qq + 1) * 128)
                pv = pvp.tile([128, 4, 128], F32, name="pv")
                n_pv = 0
                tot_pv = (qq + 1) * 4
                for b in range(nb):
                    full = (2 * b + 1) <= qq
                    jsubs = (0, 1) if full else (0,)
                    st = stp.tile([128, 2, 512], F32, name="st")
                    for jsub in jsubs:
                        j = 2 * b + jsub
                        for pl in range(2):
                            p = 2 * g + pl
                            nc.tensor.matmul(
                                st[:, jsub, pl * 256 : (pl + 1) * 256],
                                lhsT=kt_sb[p][:, j * KC : (j + 1) * KC],
                                rhs=qbd[p][:, :, qsl],
                                start=(pl == 0),
                                stop=(pl == 1),
                            )
                    pr = probs_pool.tile([128, 2, 512], BF16, name="pr")
                    if full:
                        nc.scalar.activation(
                            out=pr, in_=st,
                            func=mybir.ActivationFunctionType.Exp, scale=0.125,
                        )
                    else:
                        nc.scalar.activation(
                            out=pr[:, 0, :], in_=st[:, 0, :],
                            func=mybir.ActivationFunctionType.Exp, scale=0.125,
                        )
                    if 2 * b <= qq <= 2 * b + 1:
                        jd = qq - 2 * b
                        nc.vector.tensor_mul(pr[:, jd, :], pr[:, jd, :], maskd_sb)
                    for jsub in jsubs:
                        j = 2 * b + jsub
                        for hh in range(4):
                            h = 4 * g + hh
                            n_pv += 1
                            nc.tensor.matmul(
                                pv[:, hh, 0:65],
                                lhsT=pr[:, jsub, hh * 128 : (hh + 1) * 128],
                                rhs=vp_sb[j][:, h, :],
                                start=(n_pv == 1),
                                stop=(n_pv == tot_pv),
                            )
                # one short copy frees the pv bank; normalize off-bank from SBUF
                pvs = norm_pool.tile([128, 4, 65], BF16, name="pvs")
                nc.vector.tensor_copy(out=pvs, in_=pv[:, :, 0:65])
                rcp = norm_pool.tile([128, 4, 1], F32, name="rcp")
                nc.vector.reciprocal(out=rcp, in_=pvs[:, :, 64:65])
                for hh in range(4):
                    h = 4 * g + hh
                    nc.vector.tensor_scalar_mul(
                        ynat[qq][:, h * 64 : (h + 1) * 64],
                        pvs[:, hh, 0:64],
                        rcp[:, hh, :],
                    )

            def tails(tt):
                """Transpose y chunk tt, project through Wc, DMA out."""
                tp = scratch.tile([128, 4, 128], BF16, name="tp", tag="s")
                for c in range(NT):
                    nc.tensor.matmul(
                        tp[:, c, :],
                        lhsT=ynat[tt][:, c * 128 : (c + 1) * 128],
                        rhs=ident_sb,
                        is_transpose=True,
                        start=(c == 0),
                        stop=(c == NT - 1),
                    )
                ytc = ytc_pool.tile([128, 4, 128], BF16, name="ytc")
                nc.vector.tensor_copy(out=ytc, in_=tp)
                for co in range(2):
                    op = scratch.tile([128, 512], F32, name="op", tag="s")
                    for cb in range(NT):
                        nc.tensor.matmul(
                            op,
                            lhsT=ytc[:, cb, :],
                            rhs=wc_sb[cb][:, co * 512 : (co + 1) * 512],
                            start=(cb == 0),
                            stop=(cb == NT - 1),
                        )
                    ost = ost_pool.tile([128, 512], F32, name="ost")
                    nc.vector.tensor_copy(out=ost, in_=op)
                    nc.sync.dma_start(
                        out=out[tt * 128 : (tt + 1) * 128, co * 512 : (co + 1) * 512],
                        in_=ost,
                    )

            # ---- phase A: QK proj for head group 0 + first V chunk ----
            # (all Q units first: the wk DMAs land while Q projects)
            for kind in ("q", "k"):
                for p in (0, 1):
                    for tc4 in range(4):
                        qk_unit(p, kind, tc4)
            v_unit(0)

            # ---- merged attention stream ----
            # g0 first (needs only phase-A outputs), QK g1 + V proj
            # interleaved as PE fillers; g1's PE-heavy early chunks are
            # interleaved into g0's ACT-heavy tail; tails lag g1 by one.
            g1_units = [
                (p, kind, tc4)
                for tc4 in range(4)
                for kind in ("q", "k")
                for p in (2, 3)
            ]
            jobs = [(0, qq) for qq in range(12)]
            for i in range(4):
                jobs += [(0, 12 + i), (1, i)]
            jobs += [(1, j) for j in range(4, NQ)]
            n_g0 = 0
            for g, qq in jobs:
                if g == 0:
                    if qq + 1 < TT16:
                        v_unit(qq + 1)
                    qk_unit(*g1_units[n_g0])
                    n_g0 += 1
                    attention(0, qq)
                else:
                    attention(1, qq)
                    if qq >= 1:
                        tails(qq - 1)
            tails(NQ - 1)

    if split_waits:
        _split_sync_waits(nc)
    return nc


_NC = None


def _host_tables():
    inv_freq = 1.0 / (ROPE_BASE ** (np.arange(0, D, 2, dtype=np.float32) / D))
    t = np.arange(T, dtype=np.float32)
    freqs = np.einsum("i,j->ij", t, inv_freq)          # [T, 32]
    emb = np.concatenate([freqs, freqs], axis=-1)      # [T, 64]
    cosT = np.cos(emb).T.astype(np.float32)            # [64, T]
    sinT = np.sin(emb).T.astype(np.float32)
    cos2 = np.concatenate([cosT, cosT], axis=0)        # [128, T]
    ssin = np.concatenate([sinT, sinT], axis=0)        # [128, T]

    # causal mask on the diagonal key chunk, probs^T layout [128 keys, 128 q],
    # replicated for 4 heads
    i_ = np.arange(KC)[:, None]
    c_ = np.arange(KC)[None, :]
    m1 = (c_ >= i_).astype(np.float32)
    maskd = np.tile(m1, (1, 4))

    ident = np.eye(128, dtype=np.float32)
    # tables = [cos2 | ssin | maskd | ident]
    return np.concatenate([cos2, ssin, maskd, ident], axis=1)


def kernel(x, Wq, Wkv, Wc):
    global _NC, LAST_EXEC_NS, LAST_RESULTS
    x = np.asarray(x, dtype=np.float32)
    Wq = np.asarray(Wq, dtype=np.float32)
    Wkv = np.asarray(Wkv, dtype=np.float32)
    Wc = np.asarray(Wc, dtype=np.float32)

    if _NC is None:
        _NC = _build_nc()

    tables = _host_tables()
    bf = lambda a: np.ascontiguousarray(a).astype(NP_BF16)

    in_maps = []
    for core in range(N_CORES):
        b, hh = core // 2, core % 2
        h0 = hh * HPC
        cols = slice(h0 * D, h0 * D + CPC)
        vcols = slice(C + h0 * D, C + h0 * D + CPC)
        in_maps.append(
            {
                "xT": bf(x[b].T),
                "wqkv": bf(
                    np.concatenate(
                        [Wq[:, cols], Wkv[:, cols], Wkv[:, vcols]], axis=1
                    )
                ),
                "wc": bf(Wc[cols.start : cols.stop, :]),
                "tables": bf(tables),
            }
        )

    trace = os.environ.get("BASS_PROF", "0") == "1"
    res = run_bass_kernel_spmd(_NC, in_maps, list(range(N_CORES)), trace=trace)
    LAST_EXEC_NS = res.exec_time_ns
    LAST_RESULTS = res
    y = np.empty((B, T, C), dtype=np.float32)
    for b in range(B):
        y[b] = res.results[2 * b]["out"] + res.results[2 * b + 1]["out"]
    return y


# revision 17
# speedup vs baseline: 1.4670x; 1.0050x over previous
"""Causal self-attention (B=4, T=2048, C=1024, H=16, D=64, RoPE) on 8 trn2 cores.

Sharding: data-parallel over batch (4) x tensor-parallel over head-halves (2).
core = 2*b + hh handles batch b, heads [hh*8, hh*8+8).

Per-core kernel (all matmuls bf16 with fp32 PSUM accumulation; every matmul
operand at partition base 0 — mixed PE tile positions fault on this setup):

  - QT/KT projection in transposed layout [c_out, t] (lhsT = W column block,
    rhs = x^T), RoPE via partition-shifted 1-input DVE ops + mul/add.
    Tiles hold head pairs: rows 0-63 head 2p, rows 64-127 head 2p+1.
    RoPE'd Q is stored straight into a block-diagonal layout qbd[p]
    [128, 2, T]: block 0 rows 0-63 = head 2p (rows 64-127 zero), block 1
    rows 64-127 = head 2p+1 — scores rhs slices come from here for free.
  - V projection in natural layout [t, c_out], stored interleaved with a ones
    column per head (65 cols/head) for free softmax row-sums.
  - scores^T per (head-group g of 4, q-chunk qq of 128): batches of 2 key
    chunks land in one PSUM tile [128k, 2, 2x2x128q]; one K=128 matmul per
    (key chunk, pair) with the block-diag q slice as rhs. exp on ACT over the
    whole [128, 1024] batch (scale=0.125, no max subtraction; scores are
    O(10)); causal masking via one multiplicative bf16 mask on the diagonal
    key chunk only.
  - PV TRANSPOSED: lhsT = probs chunk [128k, 128q] (stationary), rhs = V'_h
    [128k, 65] (streamed, ones col -> col 64 = softmax denominators), out
    accumulates [128q, 65] per head over key chunks. All 4 heads of a group
    pack into ONE psum bank (single start=True on the first matmul; each
    span is first-touched exactly once while pending-zero).
  - normalize while copying out of PSUM: per-partition reciprocal of the
    denominator column, then tensor_scalar_mul psum->sbuf into y natural
    layout [t, 512].
  - y^T via PE transposes (is_transpose matmuls vs an identity, bf16 psum
    out), then row-parallel output projection -> partial [T, C] fp32 output.
Host sums the two partial outputs of each batch pair.

Phase overlap: QK proj g0 -> attention g0 (ACT exp-bound) overlapped with
QK proj g1 + V proj on PE -> attention g1 overlapped with transposes +
output projection of the previous q-chunk.
"""

import os

import numpy as np
import ml_dtypes

import concourse.bass as bass
import concourse.mybir as mybir
import concourse.tile as tile
from concourse.bass_utils import run_bass_kernel_spmd

BF16 = mybir.dt.bfloat16
F32 = mybir.dt.float32
NP_BF16 = ml_dtypes.bfloat16

B, T, C = 4, 2048, 1024
H, D = 16, 64
HPC = 8          # heads per core
CPC = HPC * D    # 512 features per core
N_CORES = 8
KC = 128         # key chunk
NQ = T // KC     # 16 q-chunks of 128
ROPE_BASE = 10000.0

LAST_EXEC_NS = None
LAST_RESULTS = None


def _split_sync_waits(nc):
    """This walrus build accepts at most one sync wait per instruction; hoist
    extra waits onto same-engine NOPs inserted immediately before."""
    ctr = 0
    for bb in nc.main_func.blocks:
        insts = bb.instructions
        new = []
        changed = False
        for inst in insts:
            si = inst.sync_info
            waits = list(si.on_wait or []) if si is not None else []
            if len(waits) > 1:
                changed = True
                for w in waits[:-1]:
                    ctr += 1
                    nop = mybir.InstNoOp(
                        name=f"waitsplit_nop_{ctr}", ins=[], outs=[],
                        engine=inst.engine,
                    )
                    nop.sync_info = mybir.SyncInfo(on_wait=[w], on_update=[])
                    new.append(nop)
                inst.sync_info = mybir.SyncInfo(
                    on_wait=[waits[-1]], on_update=list(si.on_update or [])
                )
            new.append(inst)
        if changed:
            insts[:] = new


def _build_nc(split_waits=True):
    nc = bass.Bass()

    xT = nc.dram_tensor("xT", [C, T], BF16, kind="ExternalInput")
    wqkv = nc.dram_tensor("wqkv", [C, 3 * CPC], BF16, kind="ExternalInput")
    wc = nc.dram_tensor("wc", [CPC, C], BF16, kind="ExternalInput")
    # tables = [cos2 | ssin | maskd | ident], all [128, .]
    tables = nc.dram_tensor("tables", [128, 2 * T + 4 * KC + 128], BF16,
                            kind="ExternalInput")
    out = nc.dram_tensor("out", [T, C], F32, kind="ExternalOutput")

    KB = C // 128          # 8 k-blocks over c_in
    NT = CPC // 128        # 4 head-pair tiles
    TT16 = T // 128        # 16 t tiles

    with tile.TileContext(nc) as tc:
        with (
            tc.tile_pool(name="singles", bufs=1) as singles,
            tc.tile_pool(name="xw", bufs=1) as xw,
            tc.tile_pool(name="big", bufs=1) as big,
            tc.tile_pool(name="rope", bufs=3) as rope_pool,
            tc.tile_pool(name="probs", bufs=6) as probs_pool,
            tc.tile_pool(name="normp", bufs=4) as norm_pool,
            tc.tile_pool(name="ytcp", bufs=2) as ytc_pool,
            tc.tile_pool(name="ostp", bufs=3) as ost_pool,
            tc.tile_pool(name="scratch", bufs=3, space="PSUM") as scratch,
            tc.tile_pool(name="stp", bufs=2, space="PSUM") as stp,
            tc.tile_pool(name="pvp", bufs=1, space="PSUM") as pvp,
        ):
            # ---- input DMAs: what phase A needs first (wq/xT/wk), then
            # tables, then wv/wc which are consumed later ----
            xT_sb, wq_sb, wk_sb, wv_sb = [], [], [], []
            for kb in range(KB):
                rows = slice(kb * 128, (kb + 1) * 128)
                t_ = xw.tile([128, 3 * CPC], BF16, name=f"wqkv{kb}")
                nc.sync.dma_start(out=t_[:, 0:CPC], in_=wqkv[rows, 0:CPC])
                wq_sb.append(t_[:, 0:CPC])
                wk_sb.append(t_[:, CPC : 2 * CPC])
                wv_sb.append(t_[:, 2 * CPC : 3 * CPC])
                x_ = xw.tile([128, T], BF16, name=f"xT{kb}")
                nc.sync.dma_start(out=x_, in_=xT[rows, :])
                xT_sb.append(x_)
                nc.sync.dma_start(
                    out=wk_sb[kb], in_=wqkv[rows, CPC : 2 * CPC]
                )
            tbl = singles.tile([128, 2 * T + 4 * KC + 128], BF16, name="tbl")
            nc.sync.dma_start(out=tbl, in_=tables[:])
            for kb in range(KB):
                rows = slice(kb * 128, (kb + 1) * 128)
                nc.sync.dma_start(
                    out=wv_sb[kb], in_=wqkv[rows, 2 * CPC : 3 * CPC]
                )
            cos_sb = tbl[:, 0:T]
            ssin_sb = tbl[:, T : 2 * T]
            maskd_sb = tbl[:, 2 * T : 2 * T + 4 * KC]
            ident_sb = tbl[:, 2 * T + 4 * KC : 2 * T + 4 * KC + 128]
            wcbig = xw.tile([128, NT, C], BF16, name="wcbig")
            nc.sync.dma_start(
                out=wcbig,
                in_=wc.rearrange("(cb p) c -> p cb c", p=128),
            )
            wc_sb = [wcbig[:, cb, :] for cb in range(NT)]

            # ---- persistent tiles ----
            qbd = [big.tile([128, 2, T], BF16, name=f"qbd{p}") for p in range(NT)]
            kt_sb = [big.tile([128, T], BF16, name=f"kt{p}") for p in range(NT)]
            vp_sb = [big.tile([128, HPC, 65], BF16, name=f"vp{tt}") for tt in range(TT16)]
            ynat = [big.tile([128, CPC], BF16, name=f"yn{tt}") for tt in range(TT16)]

            # zero the off-diagonal halves of the block-diag q tiles
            for p in range(NT):
                nc.gpsimd.memset(qbd[p][64:128, 0, :], 0.0)
                nc.gpsimd.memset(qbd[p][0:64, 1, :], 0.0)

            w_map = {"q": wq_sb, "k": wk_sb}

            def qk_unit(p, kind, tc4, rot_on_pool=False):
                """Project one 512-col chunk of Q^T or K^T for pair p, RoPE it."""
                ts = slice(tc4 * 512, (tc4 + 1) * 512)
                ps = scratch.tile([128, 512], F32, name="pj", tag="s")
                w_sb = w_map[kind]
                for kb in range(KB):
                    nc.tensor.matmul(
                        ps,
                        lhsT=w_sb[kb][:, p * 128 : (p + 1) * 128],
                        rhs=xT_sb[kb][:, ts],
                        start=(kb == 0),
                        stop=(kb == KB - 1),
                    )
                raw = rope_pool.tile([128, 512], BF16, name="raw")
                nc.scalar.copy(out=raw, in_=ps)
                t1 = rope_pool.tile([128, 512], BF16, name="t1")
                nc.vector.tensor_mul(t1, raw, cos_sb[:, ts])
                # rotate-half via partition-shifted single-input ops
                # (walrus allows shifted bases only for 1-input); the units
                # that overlap attention run these on the idle Pool engine
                rot_eng = nc.gpsimd if rot_on_pool else nc.vector
                rot = rope_pool.tile([128, 512], BF16, name="rot")
                for rb in (0, 64):
                    rot_eng.tensor_scalar_mul(
                        rot[rb : rb + 32, :], raw[rb + 32 : rb + 64, :], -1.0
                    )
                    rot_eng.tensor_copy(
                        out=rot[rb + 32 : rb + 64, :], in_=raw[rb : rb + 32, :]
                    )
                t2 = rope_pool.tile([128, 512], BF16, name="t2")
                nc.vector.tensor_mul(t2, rot, ssin_sb[:, ts])
                if kind == "k":
                    nc.vector.tensor_add(kt_sb[p][:, ts], t1, t2)
                else:
                    qt = rope_pool.tile([128, 512], BF16, name="qt")
                    nc.vector.tensor_add(qt, t1, t2)
                    nc.vector.tensor_copy(out=qbd[p][0:64, 0, ts], in_=qt[0:64, :])
                    nc.vector.tensor_copy(out=qbd[p][64:128, 1, ts], in_=qt[64:128, :])

            def v_unit(tt):
                """Project V for t-chunk tt into interleaved V' (65 cols/head)."""
                ps = scratch.tile([128, 512], F32, name="pj", tag="s")
                for kb in range(KB):
                    nc.tensor.matmul(
                        ps,
                        lhsT=xT_sb[kb][:, tt * 128 : (tt + 1) * 128],
                        rhs=wv_sb[kb][:, :],
                        start=(kb == 0),
                        stop=(kb == KB - 1),
                    )
                nc.vector.tensor_copy(
                    out=vp_sb[tt][:, :, 0:64],
                    in_=ps.rearrange("p (h e) -> p h e", e=64),
                )
                nc.vector.memset(vp_sb[tt][:, :, 64:65], 1.0)

            def attention(g, qq):
                """Scores^T -> exp -> mask -> transposed PV -> normalized y."""
                nb = (qq + 2) // 2
                qsl = slice(qq * 128, (qq + 1) * 128)
                pv = pvp.tile([128, 4, 128], F32, name="pv")
                n_pv = 0
                tot_pv = (qq + 1) * 4
                for b in range(nb):
                    full = (2 * b + 1) <= qq
                    jsubs = (0, 1) if full else (0,)
                    st = stp.tile([128, 2, 512], F32, name="st")
                    for jsub in jsubs:
                        j = 2 * b + jsub
                        for pl in range(2):
                            p = 2 * g + pl
                            nc.tensor.matmul(
                                st[:, jsub, pl * 256 : (pl + 1) * 256],
                                lhsT=kt_sb[p][:, j * KC : (j + 1) * KC],
                                rhs=qbd[p][:, :, qsl],
                                start=(pl == 0),
                                stop=(pl == 1),
                            )
                    pr = probs_pool.tile([128, 2, 512], BF16, name="pr")
                    if full:
                        nc.scalar.activation(
                            out=pr, in_=st,
                            func=mybir.ActivationFunctionType.Exp, scale=0.125,
                        )
                    else:
                        nc.scalar.activation(
                            out=pr[:, 0, :], in_=st[:, 0, :],
                            func=mybir.ActivationFunctionType.Exp, scale=0.125,
                        )
                    if 2 * b <= qq <= 2 * b + 1:
                        jd = qq - 2 * b
                        nc.vector.tensor_mul(pr[:, jd, :], pr[:, jd, :], maskd_sb)
                    for jsub in jsubs:
                        j = 2 * b + jsub
                        for hh in range(4):
                            h = 4 * g + hh
                            n_pv += 1
                            nc.tensor.matmul(
                                pv[:, hh, 0:65],
                                lhsT=pr[:, jsub, hh * 128 : (hh + 1) * 128],
                                rhs=vp_sb[j][:, h, :],
                                start=(n_pv == 1),
                                stop=(n_pv == tot_pv),
                            )
                # one short copy frees the pv bank; normalize off-bank from SBUF
                pvs = norm_pool.tile([128, 4, 65], BF16, name="pvs")
                nc.vector.tensor_copy(out=pvs, in_=pv[:, :, 0:65])
                rcp = norm_pool.tile([128, 4, 1], F32, name="rcp")
                nc.vector.reciprocal(out=rcp, in_=pvs[:, :, 64:65])
                for hh in range(4):
                    h = 4 * g + hh
                    nc.vector.tensor_scalar_mul(
                        ynat[qq][:, h * 64 : (h + 1) * 64],
                        pvs[:, hh, 0:64],
                        rcp[:, hh, :],
                    )

            def tails(tt):
                """Transpose y chunk tt, project through Wc, DMA out."""
                tp = scratch.tile([128, 4, 128], BF16, name="tp", tag="s")
                for c in range(NT):
                    nc.tensor.matmul(
                        tp[:, c, :],
                        lhsT=ynat[tt][:, c * 128 : (c + 1) * 128],
                        rhs=ident_sb,
                        is_transpose=True,
                        start=(c == 0),
                        stop=(c == NT - 1),
                    )
                ytc = ytc_pool.tile([128, 4, 128], BF16, name="ytc")
                nc.vector.tensor_copy(out=ytc, in_=tp)
                for co in range(2):
                    op = scratch.tile([128, 512], F32, name="op", tag="s")
                    for cb in range(NT):
                        nc.tensor.matmul(
                            op,
                            lhsT=ytc[:, cb, :],
                            rhs=wc_sb[cb][:, co * 512 : (co + 1) * 512],
                            start=(cb == 0),
                            stop=(cb == NT - 1),
                        )
                    ost = ost_pool.tile([128, 512], F32, name="ost")
                    nc.vector.tensor_copy(out=ost, in_=op)
                    nc.sync.dma_start(
                        out=out[tt * 128 : (tt + 1) * 128, co * 512 : (co + 1) * 512],
                        in_=ost,
                    )

            # ---- phase A: QK proj for head group 0 + first V chunk ----
            # (all Q units first: the wk DMAs land while Q projects)
            for kind in ("q", "k"):
                for p in (0, 1):
                    for tc4 in range(4):
                        qk_unit(p, kind, tc4)
            v_unit(0)

            # ---- merged attention stream ----
            # g0 first (needs only phase-A outputs), QK g1 + V proj
            # interleaved as PE fillers; g1's PE-heavy early chunks are
            # interleaved into g0's ACT-heavy tail; tails lag g1 by one.
            g1_units = [
                (p, kind, tc4)
                for tc4 in range(4)
                for kind in ("q", "k")
                for p in (2, 3)
            ]
            jobs = [(0, qq) for qq in range(12)]
            for i in range(4):
                jobs += [(0, 12 + i), (1, i)]
            jobs += [(1, j) for j in range(4, NQ)]
            n_g0 = 0
            for g, qq in jobs:
                if g == 0:
                    if qq + 1 < TT16:
                        v_unit(qq + 1)
                    qk_unit(*g1_units[n_g0])
                    n_g0 += 1
                    attention(0, qq)
                else:
                    attention(1, qq)
                    if qq >= 1:
                        tails(qq - 1)
            tails(NQ - 1)

    if split_waits:
        _split_sync_waits(nc)
    return nc


_NC = None


def _host_tables():
    inv_freq = 1.0 / (ROPE_BASE ** (np.arange(0, D, 2, dtype=np.float32) / D))
    t = np.arange(T, dtype=np.float32)
    freqs = np.einsum("i,j->ij", t, inv_freq)          # [T, 32]
    emb = np.concatenate([freqs, freqs], axis=-1)      # [T, 64]
    cosT = np.cos(emb).T.astype(np.float32)            # [64, T]
    sinT = np.sin(emb).T.astype(np.float32)
    cos2 = np.concatenate([cosT, cosT], axis=0)        # [128, T]
    ssin = np.concatenate([sinT, sinT], axis=0)        # [128, T]

    # causal mask on the diagonal key chunk, probs^T layout [128 keys, 128 q],
    # replicated for 4 heads
    i_ = np.arange(KC)[:, None]
    c_ = np.arange(KC)[None, :]
    m1 = (c_ >= i_).astype(np.float32)
    maskd = np.tile(m1, (1, 4))

    ident = np.eye(128, dtype=np.float32)
    # tables = [cos2 | ssin | maskd | ident]
    return np.concatenate([cos2, ssin, maskd, ident], axis=1)


def kernel(x, Wq, Wkv, Wc):
    global _NC, LAST_EXEC_NS, LAST_RESULTS
    x = np.asarray(x, dtype=np.float32)
    Wq = np.asarray(Wq, dtype=np.float32)
    Wkv = np.asarray(Wkv, dtype=np.float32)
    Wc = np.asarray(Wc, dtype=np.float32)

    if _NC is None:
        _NC = _build_nc()

    tables = _host_tables()
    bf = lambda a: np.ascontiguousarray(a).astype(NP_BF16)

    in_maps = []
    for core in range(N_CORES):
        b, hh = core // 2, core % 2
        h0 = hh * HPC
        cols = slice(h0 * D, h0 * D + CPC)
        vcols = slice(C + h0 * D, C + h0 * D + CPC)
        in_maps.append(
            {
                "xT": bf(x[b].T),
                "wqkv": bf(
                    np.concatenate(
                        [Wq[:, cols], Wkv[:, cols], Wkv[:, vcols]], axis=1
                    )
                ),
                "wc": bf(Wc[cols.start : cols.stop, :]),
                "tables": bf(tables),
            }
        )

    trace = os.environ.get("BASS_PROF", "0") == "1"
    res = run_bass_kernel_spmd(_NC, in_maps, list(range(N_CORES)), trace=trace)
    LAST_EXEC_NS = res.exec_time_ns
    LAST_RESULTS = res
    y = np.empty((B, T, C), dtype=np.float32)
    for b in range(B):
        y[b] = res.results[2 * b]["out"] + res.results[2 * b + 1]["out"]
    return y


# revision 18
# speedup vs baseline: 1.4682x; 1.0009x over previous
"""Causal self-attention (B=4, T=2048, C=1024, H=16, D=64, RoPE) on 8 trn2 cores.

Sharding: data-parallel over batch (4) x tensor-parallel over head-halves (2).
core = 2*b + hh handles batch b, heads [hh*8, hh*8+8).

Per-core kernel (all matmuls bf16 with fp32 PSUM accumulation; every matmul
operand at partition base 0 — mixed PE tile positions fault on this setup):

  - QT/KT projection in transposed layout [c_out, t] (lhsT = W column block,
    rhs = x^T), RoPE via partition-shifted 1-input DVE ops + mul/add.
    Tiles hold head pairs: rows 0-63 head 2p, rows 64-127 head 2p+1.
    RoPE'd Q is stored straight into a block-diagonal layout qbd[p]
    [128, 2, T]: block 0 rows 0-63 = head 2p (rows 64-127 zero), block 1
    rows 64-127 = head 2p+1 — scores rhs slices come from here for free.
  - V projection in natural layout [t, c_out], stored interleaved with a ones
    column per head (65 cols/head) for free softmax row-sums.
  - scores^T per (head-group g of 4, q-chunk qq of 128): batches of 2 key
    chunks land in one PSUM tile [128k, 2, 2x2x128q]; one K=128 matmul per
    (key chunk, pair) with the block-diag q slice as rhs. exp on ACT over the
    whole [128, 1024] batch (scale=0.125, no max subtraction; scores are
    O(10)); causal masking via one multiplicative bf16 mask on the diagonal
    key chunk only.
  - PV TRANSPOSED: lhsT = probs chunk [128k, 128q] (stationary), rhs = V'_h
    [128k, 65] (streamed, ones col -> col 64 = softmax denominators), out
    accumulates [128q, 65] per head over key chunks. All 4 heads of a group
    pack into ONE psum bank (single start=True on the first matmul; each
    span is first-touched exactly once while pending-zero).
  - normalize while copying out of PSUM: per-partition reciprocal of the
    denominator column, then tensor_scalar_mul psum->sbuf into y natural
    layout [t, 512].
  - y^T via PE transposes (is_transpose matmuls vs an identity, bf16 psum
    out), then row-parallel output projection -> partial [T, C] fp32 output.
Host sums the two partial outputs of each batch pair.

Phase overlap: QK proj g0 -> attention g0 (ACT exp-bound) overlapped with
QK proj g1 + V proj on PE -> attention g1 overlapped with transposes +
output projection of the previous q-chunk.
"""

import os

import numpy as np
import ml_dtypes

import concourse.bass as bass
import concourse.mybir as mybir
import concourse.tile as tile
from concourse.bass_utils import run_bass_kernel_spmd

BF16 = mybir.dt.bfloat16
F32 = mybir.dt.float32
NP_BF16 = ml_dtypes.bfloat16

B, T, C = 4, 2048, 1024
H, D = 16, 64
HPC = 8          # heads per core
CPC = HPC * D    # 512 features per core
N_CORES = 8
KC = 128         # key chunk
NQ = T // KC     # 16 q-chunks of 128
ROPE_BASE = 10000.0

LAST_EXEC_NS = None
LAST_RESULTS = None


def _split_sync_waits(nc):
    """This walrus build accepts at most one sync wait per instruction; hoist
    extra waits onto same-engine NOPs inserted immediately before."""
    ctr = 0
    for bb in nc.main_func.blocks:
        insts = bb.instructions
        new = []
        changed = False
        for inst in insts:
            si = inst.sync_info
            waits = list(si.on_wait or []) if si is not None else []
            if len(waits) > 1:
                changed = True
                for w in waits[:-1]:
                    ctr += 1
                    nop = mybir.InstNoOp(
                        name=f"waitsplit_nop_{ctr}", ins=[], outs=[],
                        engine=inst.engine,
                    )
                    nop.sync_info = mybir.SyncInfo(on_wait=[w], on_update=[])
                    new.append(nop)
                inst.sync_info = mybir.SyncInfo(
                    on_wait=[waits[-1]], on_update=list(si.on_update or [])
                )
            new.append(inst)
        if changed:
            insts[:] = new


def _build_nc(split_waits=True):
    nc = bass.Bass()

    xT = nc.dram_tensor("xT", [C, T], BF16, kind="ExternalInput")
    wqkv = nc.dram_tensor("wqkv", [C, 3 * CPC], BF16, kind="ExternalInput")
    wc = nc.dram_tensor("wc", [CPC, C], BF16, kind="ExternalInput")
    # tables = [cos2 | ssin | maskd | ident], all [128, .]
    tables = nc.dram_tensor("tables", [128, 2 * T + 4 * KC + 128], BF16,
                            kind="ExternalInput")
    out = nc.dram_tensor("out", [T, C], F32, kind="ExternalOutput")

    KB = C // 128          # 8 k-blocks over c_in
    NT = CPC // 128        # 4 head-pair tiles
    TT16 = T // 128        # 16 t tiles

    with tile.TileContext(nc) as tc:
        with (
            tc.tile_pool(name="singles", bufs=1) as singles,
            tc.tile_pool(name="xw", bufs=1) as xw,
            tc.tile_pool(name="big", bufs=1) as big,
            tc.tile_pool(name="rope", bufs=3) as rope_pool,
            tc.tile_pool(name="probs", bufs=6) as probs_pool,
            tc.tile_pool(name="normp", bufs=4) as norm_pool,
            tc.tile_pool(name="ytcp", bufs=2) as ytc_pool,
            tc.tile_pool(name="ostp", bufs=3) as ost_pool,
            tc.tile_pool(name="scratch", bufs=3, space="PSUM") as scratch,
            tc.tile_pool(name="stp", bufs=2, space="PSUM") as stp,
            tc.tile_pool(name="pvp", bufs=1, space="PSUM") as pvp,
        ):
            # ---- input DMAs: what phase A needs first (wq/xT/wk), then
            # tables, then wv/wc which are consumed later ----
            xT_sb, wq_sb, wk_sb, wv_sb = [], [], [], []
            for kb in range(KB):
                rows = slice(kb * 128, (kb + 1) * 128)
                t_ = xw.tile([128, 3 * CPC], BF16, name=f"wqkv{kb}")
                nc.sync.dma_start(out=t_[:, 0:CPC], in_=wqkv[rows, 0:CPC])
                wq_sb.append(t_[:, 0:CPC])
                wk_sb.append(t_[:, CPC : 2 * CPC])
                wv_sb.append(t_[:, 2 * CPC : 3 * CPC])
                x_ = xw.tile([128, T], BF16, name=f"xT{kb}")
                nc.sync.dma_start(out=x_, in_=xT[rows, :])
                xT_sb.append(x_)
                nc.sync.dma_start(
                    out=wk_sb[kb], in_=wqkv[rows, CPC : 2 * CPC]
                )
            tbl = singles.tile([128, 2 * T + 4 * KC + 128], BF16, name="tbl")
            nc.sync.dma_start(out=tbl, in_=tables[:])
            for kb in range(KB):
                rows = slice(kb * 128, (kb + 1) * 128)
                nc.sync.dma_start(
                    out=wv_sb[kb], in_=wqkv[rows, 2 * CPC : 3 * CPC]
                )
            cos_sb = tbl[:, 0:T]
            ssin_sb = tbl[:, T : 2 * T]
            maskd_sb = tbl[:, 2 * T : 2 * T + 4 * KC]
            ident_sb = tbl[:, 2 * T + 4 * KC : 2 * T + 4 * KC + 128]
            wcbig = xw.tile([128, NT, C], BF16, name="wcbig")
            nc.sync.dma_start(
                out=wcbig,
                in_=wc.rearrange("(cb p) c -> p cb c", p=128),
            )
            wc_sb = [wcbig[:, cb, :] for cb in range(NT)]

            # ---- persistent tiles ----
            qbd = [big.tile([128, 2, T], BF16, name=f"qbd{p}") for p in range(NT)]
            kt_sb = [big.tile([128, T], BF16, name=f"kt{p}") for p in range(NT)]
            vp_sb = [big.tile([128, HPC, 65], BF16, name=f"vp{tt}") for tt in range(TT16)]
            ynat = [big.tile([128, CPC], BF16, name=f"yn{tt}") for tt in range(TT16)]

            # zero the off-diagonal halves of the block-diag q tiles
            for p in range(NT):
                nc.gpsimd.memset(qbd[p][64:128, 0, :], 0.0)
                nc.gpsimd.memset(qbd[p][0:64, 1, :], 0.0)

            w_map = {"q": wq_sb, "k": wk_sb}

            def qk_unit(p, kind, tc4, rot_on_pool=False):
                """Project one 512-col chunk of Q^T or K^T for pair p, RoPE it."""
                ts = slice(tc4 * 512, (tc4 + 1) * 512)
                ps = scratch.tile([128, 512], F32, name="pj", tag="s")
                w_sb = w_map[kind]
                for kb in range(KB):
                    nc.tensor.matmul(
                        ps,
                        lhsT=w_sb[kb][:, p * 128 : (p + 1) * 128],
                        rhs=xT_sb[kb][:, ts],
                        start=(kb == 0),
                        stop=(kb == KB - 1),
                    )
                raw = rope_pool.tile([128, 512], BF16, name="raw")
                nc.scalar.copy(out=raw, in_=ps)
                t1 = rope_pool.tile([128, 512], BF16, name="t1")
                nc.vector.tensor_mul(t1, raw, cos_sb[:, ts])
                # rotate-half via partition-shifted single-input ops
                # (walrus allows shifted bases only for 1-input); the units
                # that overlap attention run these on the idle Pool engine
                rot_eng = nc.gpsimd if rot_on_pool else nc.vector
                rot = rope_pool.tile([128, 512], BF16, name="rot")
                for rb in (0, 64):
                    rot_eng.tensor_scalar_mul(
                        rot[rb : rb + 32, :], raw[rb + 32 : rb + 64, :], -1.0
                    )
                    rot_eng.tensor_copy(
                        out=rot[rb + 32 : rb + 64, :], in_=raw[rb : rb + 32, :]
                    )
                t2 = rope_pool.tile([128, 512], BF16, name="t2")
                nc.vector.tensor_mul(t2, rot, ssin_sb[:, ts])
                if kind == "k":
                    nc.vector.tensor_add(kt_sb[p][:, ts], t1, t2)
                else:
                    qt = rope_pool.tile([128, 512], BF16, name="qt")
                    nc.vector.tensor_add(qt, t1, t2)
                    nc.vector.tensor_copy(out=qbd[p][0:64, 0, ts], in_=qt[0:64, :])
                    nc.vector.tensor_copy(out=qbd[p][64:128, 1, ts], in_=qt[64:128, :])

            def v_unit(tt):
                """Project V for t-chunk tt into interleaved V' (65 cols/head)."""
                ps = scratch.tile([128, 512], F32, name="pj", tag="s")
                for kb in range(KB):
                    nc.tensor.matmul(
                        ps,
                        lhsT=xT_sb[kb][:, tt * 128 : (tt + 1) * 128],
                        rhs=wv_sb[kb][:, :],
                        start=(kb == 0),
                        stop=(kb == KB - 1),
                    )
                nc.scalar.copy(
                    out=vp_sb[tt][:, :, 0:64],
                    in_=ps.rearrange("p (h e) -> p h e", e=64),
                )
                nc.vector.memset(vp_sb[tt][:, :, 64:65], 1.0)

            def attention(g, qq):
                """Scores^T -> exp -> mask -> transposed PV -> normalized y."""
                nb = (qq + 2) // 2
                qsl = slice(qq * 128, (qq + 1) * 128)
                pv = pvp.tile([128, 4, 128], F32, name="pv")
                n_pv = 0
                tot_pv = (qq + 1) * 4
                for b in range(nb):
                    full = (2 * b + 1) <= qq
                    jsubs = (0, 1) if full else (0,)
                    st = stp.tile([128, 2, 512], F32, name="st")
                    for jsub in jsubs:
                        j = 2 * b + jsub
                        for pl in range(2):
                            p = 2 * g + pl
                            nc.tensor.matmul(
                                st[:, jsub, pl * 256 : (pl + 1) * 256],
                                lhsT=kt_sb[p][:, j * KC : (j + 1) * KC],
                                rhs=qbd[p][:, :, qsl],
                                start=(pl == 0),
                                stop=(pl == 1),
                            )
                    pr = probs_pool.tile([128, 2, 512], BF16, name="pr")
                    if full:
                        nc.scalar.activation(
                            out=pr, in_=st,
                            func=mybir.ActivationFunctionType.Exp, scale=0.125,
                        )
                    else:
                        nc.scalar.activation(
                            out=pr[:, 0, :], in_=st[:, 0, :],
                            func=mybir.ActivationFunctionType.Exp, scale=0.125,
                        )
                    if 2 * b <= qq <= 2 * b + 1:
                        jd = qq - 2 * b
                        nc.vector.tensor_mul(pr[:, jd, :], pr[:, jd, :], maskd_sb)
                    for jsub in jsubs:
                        j = 2 * b + jsub
                        for hh in range(4):
                            h = 4 * g + hh
                            n_pv += 1
                            nc.tensor.matmul(
                                pv[:, hh, 0:65],
                                lhsT=pr[:, jsub, hh * 128 : (hh + 1) * 128],
                                rhs=vp_sb[j][:, h, :],
                                start=(n_pv == 1),
                                stop=(n_pv == tot_pv),
                            )
                # one short copy frees the pv bank; normalize off-bank from SBUF
                pvs = norm_pool.tile([128, 4, 65], BF16, name="pvs")
                nc.vector.tensor_copy(out=pvs, in_=pv[:, :, 0:65])
                rcp = norm_pool.tile([128, 4, 1], F32, name="rcp")
                nc.vector.reciprocal(out=rcp, in_=pvs[:, :, 64:65])
                for hh in range(4):
                    h = 4 * g + hh
                    nc.vector.tensor_scalar_mul(
                        ynat[qq][:, h * 64 : (h + 1) * 64],
                        pvs[:, hh, 0:64],
                        rcp[:, hh, :],
                    )

            def tails(tt):
                """Transpose y chunk tt, project through Wc, DMA out."""
                tp = scratch.tile([128, 4, 128], BF16, name="tp", tag="s")
                for c in range(NT):
                    nc.tensor.matmul(
                        tp[:, c, :],
                        lhsT=ynat[tt][:, c * 128 : (c + 1) * 128],
                        rhs=ident_sb,
                        is_transpose=True,
                        start=(c == 0),
                        stop=(c == NT - 1),
                    )
                ytc = ytc_pool.tile([128, 4, 128], BF16, name="ytc")
                nc.vector.tensor_copy(out=ytc, in_=tp)
                for co in range(2):
                    op = scratch.tile([128, 512], F32, name="op", tag="s")
                    for cb in range(NT):
                        nc.tensor.matmul(
                            op,
                            lhsT=ytc[:, cb, :],
                            rhs=wc_sb[cb][:, co * 512 : (co + 1) * 512],
                            start=(cb == 0),
                            stop=(cb == NT - 1),
                        )
                    ost = ost_pool.tile([128, 512], F32, name="ost")
                    nc.vector.tensor_copy(out=ost, in_=op)
                    nc.sync.dma_start(
                        out=out[tt * 128 : (tt + 1) * 128, co * 512 : (co + 1) * 512],
                        in_=ost,
                    )

            # ---- phase A: QK proj for head group 0 + first V chunk ----
            # (all Q units first: the wk DMAs land while Q projects)
            for kind in ("q", "k"):
                for p in (0, 1):
                    for tc4 in range(4):
                        qk_unit(p, kind, tc4)
            v_unit(0)

            # ---- merged attention stream ----
            # g0 first (needs only phase-A outputs), QK g1 + V proj
            # interleaved as PE fillers; g1's PE-heavy early chunks are
            # interleaved into g0's ACT-heavy tail; tails lag g1 by one.
            g1_units = [
                (p, kind, tc4)
                for tc4 in range(4)
                for kind in ("q", "k")
                for p in (2, 3)
            ]
            jobs = [(0, qq) for qq in range(12)]
            for i in range(4):
                jobs += [(0, 12 + i), (1, i)]
            jobs += [(1, j) for j in range(4, NQ)]
            n_g0 = 0
            for g, qq in jobs:
                if g == 0:
                    if qq + 1 < TT16:
                        v_unit(qq + 1)
                    qk_unit(*g1_units[n_g0])
                    n_g0 += 1
                    attention(0, qq)
                else:
                    attention(1, qq)
                    if qq >= 1:
                        tails(qq - 1)
            tails(NQ - 1)

    if split_waits:
        _split_sync_waits(nc)
    return nc


_NC = None


def _host_tables():
    inv_freq = 1.0 / (ROPE_BASE ** (np.arange(0, D, 2, dtype=np.float32) / D))
    t = np.arange(T, dtype=np.float32)
    freqs = np.einsum("i,j->ij", t, inv_freq)          # [T, 32]
    emb = np.concatenate([freqs, freqs], axis=-1)      # [T, 64]
    cosT = np.cos(emb).T.astype(np.float32)            # [64, T]
    sinT = np.sin(emb).T.astype(np.float32)
    cos2 = np.concatenate([cosT, cosT], axis=0)        # [128, T]
    ssin = np.concatenate([sinT, sinT], axis=0)        # [128, T]

    # causal mask on the diagonal key chunk, probs^T layout [128 keys, 128 q],
    # replicated for 4 heads
    i_ = np.arange(KC)[:, None]
    c_ = np.arange(KC)[None, :]
    m1 = (c_ >= i_).astype(np.float32)
    maskd = np.tile(m1, (1, 4))

    ident = np.eye(128, dtype=np.float32)
    # tables = [cos2 | ssin | maskd | ident]
    return np.concatenate([cos2, ssin, maskd, ident], axis=1)


def kernel(x, Wq, Wkv, Wc):
    global _NC, LAST_EXEC_NS, LAST_RESULTS
    x = np.asarray(x, dtype=np.float32)
    Wq = np.asarray(Wq, dtype=np.float32)
    Wkv = np.asarray(Wkv, dtype=np.float32)
    Wc = np.asarray(Wc, dtype=np.float32)

    if _NC is None:
        _NC = _build_nc()

    tables = _host_tables()
    bf = lambda a: np.ascontiguousarray(a).astype(NP_BF16)

    in_maps = []
    for core in range(N_CORES):
        b, hh = core // 2, core % 2
        h0 = hh * HPC
        cols = slice(h0 * D, h0 * D + CPC)
        vcols = slice(C + h0 * D, C + h0 * D + CPC)
        in_maps.append(
            {
                "xT": bf(x[b].T),
                "wqkv": bf(
                    np.concatenate(
                        [Wq[:, cols], Wkv[:, cols], Wkv[:, vcols]], axis=1
                    )
                ),
                "wc": bf(Wc[cols.start : cols.stop, :]),
                "tables": bf(tables),
            }
        )

    trace = os.environ.get("BASS_PROF", "0") == "1"
    res = run_bass_kernel_spmd(_NC, in_maps, list(range(N_CORES)), trace=trace)
    LAST_EXEC_NS = res.exec_time_ns
    LAST_RESULTS = res
    y = np.empty((B, T, C), dtype=np.float32)
    for b in range(B):
        y[b] = res.results[2 * b]["out"] + res.results[2 * b + 1]["out"]
    return y
